# revision 1
# baseline (speedup 1.0000x reference)
"""Multi-head attention on 8 Trainium2 NeuronCores.

Problem: B=2, T=2048, D=1024, H=16 heads (dh=64), int 0/1 attention mask.

Sharding (hardcoded): core c -> batch b = c//4, head block hb = c%4
(4 heads = 256 cols per core). Wq/Wk/Wv column-sharded, Wo row-sharded;
each core returns a partial [T, D] output, host sums the 4 partials per
batch and adds bo.

Per-core kernel (all matmul inputs fp16, fp32 accumulation):
  phase 1: Q^T = (Wq_c)^T X^T (scaled+bias via ACT), K^T likewise,
           V = X Wv_c + bv (bias via K=1 ones matmul), V augmented with a
           ones column per head (denominator trick).
  phase 2 (per head, per 128-row k-tile of the T axis):
           S^T[k,q] = K_h Q_h^T  ->  E = exp(S^T)  ->  E *= mask^T tile
           U_aug^T += V_aug_h[k-tile]^T E            (PSUM accum over k)
           V_aug has the ones column replicated 64x, so U_aug^T rows
           64:128 all hold the softmax denominator row -- the matmul
           itself broadcasts it; normalize = reciprocal + one multiply.
  phase 3: O_partial = Hcat^T.T Wo_c  -> DMA out as fp16 (summed in f32
           on the host).

No max-subtraction is needed: scores are O(1) (exp range ~e^-6..e^6) and
softmax(x) == softmax(x - max) exactly in the masked-multiplicative form
E = exp(S) * m / sum(exp(S) * m).
"""
import contextlib
import os
import sys
import time

# more robust against a previously wedged device; must be set before the
# jax/axon backend initializes
os.environ.setdefault("NEURON_RT_RESET_CORES", "1")

if "/opt/trn_rl_repo" not in sys.path:
    sys.path.insert(0, "/opt/trn_rl_repo")

import numpy as np

import concourse.bass as bass  # noqa: F401  (import keeps bass registered)
from concourse import bacc
import concourse.mybir as mybir
import concourse.tile as tile
from concourse.bass_utils import run_bass_kernel_spmd

f32 = mybir.dt.float32
f16 = mybir.dt.float16
AF = mybir.ActivationFunctionType

B, T, D, H = 2, 2048, 1024, 16
DH = 64                 # head dim
NHC = 4                 # heads per core
C = NHC * DH            # 256 columns per core
KD = D // 128           # 8 contraction tiles over D
KT = T // 128           # 16 k-tiles over T
QC = T // 512           # 4 q chunks of 512
NCORES = 8
SCALE = DH ** -0.5      # 0.125

_CACHE = {}


def _build(repeat=1):
    nc = bacc.Bacc()
    xt = nc.declare_dram_parameter("xt", [D, T], f16, isOutput=False)
    wq = nc.declare_dram_parameter("wq", [D, C], f16, isOutput=False)
    wk = nc.declare_dram_parameter("wk", [D, C], f16, isOutput=False)
    wv = nc.declare_dram_parameter("wv", [D, C], f16, isOutput=False)
    wo = nc.declare_dram_parameter("wo", [C, D], f16, isOutput=False)
    maskt = nc.declare_dram_parameter("maskt", [T, T], f16, isOutput=False)
    bqs = nc.declare_dram_parameter("bqs", [C], f32, isOutput=False)
    bks = nc.declare_dram_parameter("bks", [C], f32, isOutput=False)
    bvr = nc.declare_dram_parameter("bvr", [1, C], f16, isOutput=False)
    out = nc.declare_dram_parameter("out", [T, D], f16, isOutput=True)

    with tile.TileContext(nc) as tc:
        loop_ctx = tc.For_i(0, repeat, 1) if repeat > 1 else contextlib.nullcontext()
        with (
            loop_ctx,
            tc.tile_pool(name="persist", bufs=1) as pp,
            tc.tile_pool(name="e", bufs=8) as ep,
            tc.tile_pool(name="osb", bufs=6) as op_,
            tc.tile_pool(name="small", bufs=1) as sp,
        ):
            xt_sb = pp.tile([128, KD, T], f16)
            wq_sb = pp.tile([128, KD, C], f16)
            wk_sb = pp.tile([128, KD, C], f16)
            wv_sb = pp.tile([128, KD, C], f16)
            wo_sb = pp.tile([128, C // 128, D], f16)
            mk_sb = pp.tile([128, KT, T], f16)
            qt_sb = pp.tile([128, C // 128, T], f16)
            kt_sb = pp.tile([128, C // 128, T], f16)
            v_sb = pp.tile([128, KT, NHC * 2 * DH], f16)
            hc_sb = pp.tile([128, C // 128, T], f16)
            bq_sb = pp.tile([128, C // 128], f32)
            bk_sb = pp.tile([128, C // 128], f32)
            bv_sb = pp.tile([1, C], f16)
            ones128 = pp.tile([1, 128], f16)

            # ---- input DMAs ----
            # Weights + xt interleaved per k-tile on HWDGE so phase-1 matmuls
            # start as soon as the first tiles land; mask tiles (phase 2
            # only) go on the SWDGE queues via gpsimd.
            xt_r = xt.rearrange("(kd p) t -> p kd t", p=128)
            wq_r = wq.rearrange("(kd p) c -> p kd c", p=128)
            wk_r = wk.rearrange("(kd p) c -> p kd c", p=128)
            wv_r = wv.rearrange("(kd p) c -> p kd c", p=128)
            # startup-critical order: wq, then qc0's xt in 2-ktile pieces so
            # the first projection group can chase the DMA wave
            # SWDGE lane is idle at startup (mask DMAs come later): put the
            # first weight tiles there so they land in parallel with xt
            nc.gpsimd.dma_start(out=wq_sb[:, 0:2, :], in_=wq_r[:, 0:2, :])
            nc.gpsimd.dma_start(out=wq_sb[:, 2:KD, :], in_=wq_r[:, 2:KD, :])
            for kd2 in range(0, KD, 2):
                nc.sync.dma_start(
                    out=xt_sb[:, kd2 : kd2 + 2, 0:512],
                    in_=xt_r[:, kd2 : kd2 + 2, 0:512],
                )
            nc.gpsimd.dma_start(out=bq_sb, in_=bqs.rearrange("(m p) -> p m", p=128))
            nc.sync.dma_start(out=wk_sb, in_=wk_r)
            nc.sync.dma_start(out=bk_sb, in_=bks.rearrange("(m p) -> p m", p=128))
            nc.sync.dma_start(out=wv_sb, in_=wv_r)
            nc.sync.dma_start(out=bv_sb, in_=bvr[:, :])
            for qc in range(1, QC):
                nc.sync.dma_start(
                    out=xt_sb[:, :, qc * 512 : (qc + 1) * 512],
                    in_=xt_r[:, :, qc * 512 : (qc + 1) * 512],
                )
            nc.sync.dma_start(out=wo_sb, in_=wo.rearrange("(m p) d -> p m d", p=128))
            nc.vector.memset(ones128, 1.0)
            v4 = v_sb.rearrange("p kt (h x) -> p kt h x", x=2 * DH)
            nc.vector.memset(v4[:, :, :, DH:], 1.0)

            # ---- phase 1: projections ----
            with tc.tile_pool(name="ps1", bufs=2, space="PSUM") as ps1:
                for qc in range(QC):
                    for w_sb, b_sb, dst, scale in (
                        (wq_sb, bq_sb, qt_sb, SCALE),
                        (wk_sb, bk_sb, kt_sb, 1.0),
                    ):
                        for m in range(C // 128):
                            pt = ps1.tile([128, 512], f32, tag="p")
                            for kd in range(KD):
                                nc.tensor.matmul(
                                    pt,
                                    w_sb[:, kd, m * 128 : (m + 1) * 128],
                                    xt_sb[:, kd, qc * 512 : (qc + 1) * 512],
                                    start=(kd == 0),
                                    stop=(kd == KD - 1),
                                )
                            nc.scalar.activation(
                                dst[:, m, qc * 512 : (qc + 1) * 512],
                                pt,
                                AF.Identity,
                                bias=b_sb[:, m : m + 1],
                                scale=scale,
                            )
                    for tt in range(4):
                        t = qc * 4 + tt
                        pv = ps1.tile([128, C], f32, tag="v")
                        for kd in range(KD):
                            nc.tensor.matmul(
                                pv,
                                xt_sb[:, kd, t * 128 : (t + 1) * 128],
                                wv_sb[:, kd, :],
                                start=(kd == 0),
                                stop=False,
                            )
                        nc.tensor.matmul(pv, ones128, bv_sb, start=False, stop=True)
                        nc.vector.tensor_copy(
                            v4[:, t, :, 0:DH],
                            pv.rearrange("p (h x) -> p h x", x=DH),
                        )

            # mask tiles are only needed in phase 2; emitting their DMAs
            # after phase 1 keeps the startup HWDGE/bandwidth free for xt+w
            mk_r = maskt.rearrange("(kt p) t -> p kt t", p=128)
            for kt in range(KT):
                nc.gpsimd.dma_start(out=mk_sb[:, kt, :], in_=mk_r[:, kt, :])

            # ---- phase 2: attention per head ----
            with (
                tc.tile_pool(name="ps_s", bufs=2, space="PSUM") as pss,
                tc.tile_pool(name="ps_u", bufs=1, space="PSUM") as psu,
            ):
                def s_matmuls(h, kt):
                    m, p0 = h // 2, (h % 2) * 64
                    halves = []
                    ctx2 = tc.high_priority(offset=24)
                    ctx2.__enter__()
                    for half in range(2):
                        st = pss.tile([128, 1024], f32, tag="s")
                        for sub in range(2):
                            qc = half * 2 + sub
                            nc.tensor.matmul(
                                st[:, sub * 512 : (sub + 1) * 512],
                                kt_sb[p0 : p0 + 64, m, kt * 128 : (kt + 1) * 128],
                                qt_sb[p0 : p0 + 64, m, qc * 512 : (qc + 1) * 512],
                                start=True,
                                stop=True,
                            )
                        halves.append(st)
                    ctx2.__exit__(None, None, None)
                    return halves

                # software pipeline: S matmuls for step i+1 are emitted on PE
                # before the (DVE-gated) U matmuls of step i, so ACT's exp
                # stream never waits behind PE head-of-line blocking.
                steps = [(h, kt) for h in range(NHC) for kt in range(KT)]
                st_next = s_matmuls(*steps[0])
                u = None
                for i, (h, kt) in enumerate(steps):
                    m, p0 = h // 2, (h % 2) * 64
                    if kt == 0:
                        u = psu.tile([2 * DH, T], f32, tag="u")
                    st_cur = st_next
                    if i + 1 < len(steps):
                        st_next = s_matmuls(*steps[i + 1])
                    e = ep.tile([128, T], f16)
                    for half in range(2):
                        nc.scalar.activation(
                            e[:, half * 1024 : (half + 1) * 1024],
                            st_cur[half],
                            AF.Exp,
                        )
                    nc.vector.tensor_mul(e, e, mk_sb[:, kt, :])
                    for qc in range(QC):
                        nc.tensor.matmul(
                            u[:, qc * 512 : (qc + 1) * 512],
                            v_sb[:, kt, h * 2 * DH : (h + 1) * 2 * DH],
                            e[:, qc * 512 : (qc + 1) * 512],
                            start=(kt == 0),
                            stop=(kt == KT - 1),
                        )
                    if kt == KT - 1:
                        with tc.high_priority(offset=40):
                            recb = sp.tile([64, T], f32, tag="recb")
                            for half in range(2):
                                sl = slice(half * 1024, (half + 1) * 1024)
                                nc.vector.reciprocal(recb[:, sl], u[DH : 2 * DH, sl])
                                nc.vector.tensor_mul(
                                    hc_sb[p0 : p0 + 64, m, sl],
                                    u[0:DH, sl],
                                    recb[:, sl],
                                )

            # ---- phase 3: output projection ----
            with tc.tile_pool(name="ps_o", bufs=8, space="PSUM") as pso:
                for t in range(KT):
                    ot = op_.tile([128, 1024], f16)
                    for n in range(2):
                        po = pso.tile([128, 512], f32, tag="o")
                        for m in range(C // 128):
                            nc.tensor.matmul(
                                po,
                                hc_sb[:, m, t * 128 : (t + 1) * 128],
                                wo_sb[:, m, n * 512 : (n + 1) * 512],
                                start=(m == 0),
                                stop=(m == C // 128 - 1),
                            )
                        if (t * 2 + n) % 2 == 0:
                            nc.vector.tensor_copy(ot[:, n * 512 : (n + 1) * 512], po)
                        else:
                            nc.scalar.activation(
                                ot[:, n * 512 : (n + 1) * 512], po, AF.Identity
                            )
                    nc.sync.dma_start(
                        out=out[t * 128 : (t + 1) * 128, :],
                        in_=ot,
                    )
    nc.compile()
    return nc


def _get_nc(repeat=1):
    key = ("nc", repeat)
    if key not in _CACHE:
        _CACHE[key] = _build(repeat)
    return _CACHE[key]


def _prep_core_inputs(c, x, mask, Wq, bq, Wk, bk, Wv, bv, Wo):
    b, hb = divmod(c, NCORES // B)
    sl = slice(hb * C, (hb + 1) * C)
    return {
        "xt": np.ascontiguousarray(x[b].T).astype(np.float16),
        "wq": np.ascontiguousarray(Wq[:, sl]).astype(np.float16),
        "wk": np.ascontiguousarray(Wk[:, sl]).astype(np.float16),
        "wv": np.ascontiguousarray(Wv[:, sl]).astype(np.float16),
        "wo": np.ascontiguousarray(Wo[sl, :]).astype(np.float16),
        "maskt": np.ascontiguousarray(mask[b].T).astype(np.float16),
        "bqs": (bq[sl] * SCALE).astype(np.float32),
        "bks": bk[sl].astype(np.float32),
        "bvr": bv[sl].astype(np.float16).reshape(1, C),
    }


def kernel(
    inputs, mask, Wq, bq, Wk, bk, Wv, bv, Wo, bo,
    _trace=False, _trace_kwargs=None, _repeat=1,
):
    x = np.asarray(inputs, dtype=np.float32)
    mask = np.asarray(mask)
    Wq, bq = np.asarray(Wq, np.float32), np.asarray(bq, np.float32)
    Wk, bk = np.asarray(Wk, np.float32), np.asarray(bk, np.float32)
    Wv, bv = np.asarray(Wv, np.float32), np.asarray(bv, np.float32)
    Wo, bo = np.asarray(Wo, np.float32), np.asarray(bo, np.float32)

    nc = _get_nc(_repeat)
    in_maps = [
        _prep_core_inputs(c, x, mask, Wq, bq, Wk, bk, Wv, bv, Wo)
        for c in range(NCORES)
    ]
    last_err = None
    for attempt in range(3):
        try:
            res = run_bass_kernel_spmd(
                nc,
                in_maps,
                list(range(NCORES)),
                trace=_trace,
                **(_trace_kwargs or {}),
            )
            break
        except Exception as e:  # wedged device etc. -- retry
            last_err = e
            time.sleep(3.0)
    else:
        raise last_err
    out = np.empty((B, T, D), np.float32)
    per_b = NCORES // B
    for b in range(B):
        acc = res.results[b * per_b]["out"].astype(np.float32)
        for j in range(1, per_b):
            acc = acc + res.results[b * per_b + j]["out"].astype(np.float32)
        out[b] = acc + bo[None, :]
    if _trace:
        kernel.last_results = res
    return out



# revision 66
# speedup vs baseline: 1.1481x; 1.1481x over previous
"""Multi-head attention on 8 Trainium2 NeuronCores, fp8-DoubleRow edition.

Problem: B=2, T=2048, D=1024, H=16 heads (dh=64), int 0/1 attention mask.

Sharding (hardcoded): core c -> batch b = c//4, head block hb = c%4
(4 heads = 256 cols per core). Wq/Wk/Wv column-sharded, Wo row-sharded;
each core returns a partial [T, D] output, host sums the 4 partials per
batch and adds bo.

Precision strategy: every matmul except U (= attn @ V) runs as fp8e4
DoubleRow (0.5 cyc/row, 2 k-subtiles per instr) with full value+residual
correction, so accuracy stays fp16-class:
  A @ B == (A8 + dA8)(B8 + dB8) up to ~0.4% second-order residue, where
  X8 = fp8(X), dX8 = fp8(X - X8).
- projections (Q/K/V): 3-term DR (A8B8 | dA8.B8 | A8.dB8), kd-pair packed,
  x and W pairs prepped host-side; bias added via fp16 ones-matmul.
- S = K^T Q per head (contraction dh=64): ONE DR instr per output tile:
  stationary = [K8; dK8] stacked in partitions (128 rows), sub0 moving =
  [Q8; Q8], sub1 moving = [dQ8; dQ8]  =>  (K8+dK8)^T (Q8+dQ8) exactly.
- output proj: 3-term DR over the c-halves (contraction C=256 packed).
- U stays fp16: E = exp(S) can't be residual-corrected without an extra
  131k-row pass (ACT can only emit one output), and raw-fp8 E costs
  2.3e-2 rel err (> the 2e-2 gate).

Phase 2 runs per (head, q-half): S tiles [128,1024] double-buffered +
U accumulator [128,1024] fit PSUM exactly; exp is ACT-bound (~133us) and
everything else hides under it.  Engines drain their queues in emission
order, so the remaining phase-1 work (m=1 projections) is emitted
piecewise inside the step loop ("injections"), phase 3a (t-blocks 0-7)
is injected once every head's q-half 0 is normalized, and only phase 3b
trails the last attention block.  The denominator ones-columns carry
1/SH so the normalize multiply directly yields h*SH for the fp8 pair
(plain copy/sub, legal on gpsimd: GPSIMD cannot access PSUM, and Pool
TensorScalarPtr is rejected by codegen).

No max-subtraction is needed: scores are O(1) (exp range ~e^-6..e^6) and
softmax(x) == softmax(x - max) exactly in the masked-multiplicative form
E = exp(S) * m / sum(exp(S) * m).
"""
import contextlib
import os
import sys
import time

os.environ.setdefault("NEURON_RT_RESET_CORES", "1")

if "/opt/trn_rl_repo" not in sys.path:
    sys.path.insert(0, "/opt/trn_rl_repo")

import numpy as np
import ml_dtypes

import concourse.bass as bass  # noqa: F401  (import keeps bass registered)
from concourse import bacc
import concourse.mybir as mybir
import concourse.tile as tile
from concourse.bass_utils import run_bass_kernel_spmd

f32 = mybir.dt.float32
f16 = mybir.dt.float16
f8 = mybir.dt.float8e4
AF = mybir.ActivationFunctionType
DR = mybir.MatmulPerfMode.DoubleRow

NP8 = ml_dtypes.float8_e4m3

B, T, D, H = 2, 2048, 1024, 16
DH = 64                 # head dim
NHC = 4                 # heads per core
C = NHC * DH            # 256 columns per core
KD = D // 128           # 8 contraction tiles over D
KT = T // 128           # 16 k-tiles over T
NCORES = 8
SCALE = DH ** -0.5      # 0.125

# power-of-two fp8 range scaling: e4m3 normals span ~[2**-6, 240], and the
# raw tensors here (W ~ N(0, 1/D), q ~ 0.125, h ~ 0.05) sit deep in its
# subnormal floor, which destroys the value+residual pairs.  Each fp8
# tensor is stored pre-scaled into e4m3's sweet spot; the compensations
# ride existing scale slots (ACT activation scale, DVE/gpsimd
# tensor_scalar ops), so they cost nothing extra.
SX = 2.0 ** 5        # x
SWQ = 2.0 ** 13      # Wq * SCALE
SWK = 2.0 ** 10      # Wk / Wv / Wo
SQ = 2.0 ** 8        # q pair
SK = 2.0 ** 5        # k pair
SH = 2.0 ** 8        # h pair
SQK_PS = 2.0 ** -10  # psum(q*2^18) -> q*2^8; psum(k*2^15) -> k*2^5
SV_PS = 2.0 ** -15   # psum(v*2^15) -> v
SEXP = 2.0 ** -13    # S psum carries scores * 2^(8+5)
SOUT = 2.0 ** -18    # phase-3 psum carries out * 2^(8+10)

_CACHE = {}


def _build(repeat=1, dbg=False):
    nc = bacc.Bacc()
    xp = nc.declare_dram_parameter("xp", [2 * D, T], f8, isOutput=False)
    wqp = nc.declare_dram_parameter("wqp", [2 * D, 2 * C], f8, isOutput=False)
    wkp = nc.declare_dram_parameter("wkp", [2 * D, C], f8, isOutput=False)
    wvp = nc.declare_dram_parameter("wvp", [2 * D, C], f8, isOutput=False)
    wop = nc.declare_dram_parameter("wop", [2 * C, D], f8, isOutput=False)
    bq16 = nc.declare_dram_parameter("bq16", [1, 2 * C], f16, isOutput=False)
    bk16 = nc.declare_dram_parameter("bk16", [1, C], f16, isOutput=False)
    bv16 = nc.declare_dram_parameter("bv16", [1, C], f16, isOutput=False)
    maskt = nc.declare_dram_parameter("maskt", [T, T], f16, isOutput=False)
    out = nc.declare_dram_parameter("out", [T, D], f16, isOutput=True)
    if dbg:
        qst_o = nc.declare_dram_parameter("qst_o", [128, NHC, 2, T], f8, isOutput=True)
        kst_o = nc.declare_dram_parameter("kst_o", [128, NHC, T], f8, isOutput=True)
        v_o = nc.declare_dram_parameter("v_o", [128, KT, NHC * 2 * DH], f16, isOutput=True)
        hc_o = nc.declare_dram_parameter("hc_o", [128, 2, T], f8, isOutput=True)
        dhc_o = nc.declare_dram_parameter("dhc_o", [128, 2, T], f8, isOutput=True)

    xp_r = xp.rearrange("(g kd p) t -> p (g kd) t", p=128, g=2)
    wqp_r = wqp.rearrange("(g kd p) c -> p (g kd) c", p=128, g=2)
    wkp_r = wkp.rearrange("(g kd p) c -> p (g kd) c", p=128, g=2)
    wvp_r = wvp.rearrange("(g kd p) c -> p (g kd) c", p=128, g=2)
    wop_r = wop.rearrange("(g h p) d -> p (g h) d", p=128, g=2)
    mk_r = maskt.rearrange("(kt p) t -> p kt t", p=128)

    with tile.TileContext(nc) as tc:
        loop_ctx = tc.For_i(0, repeat, 1) if repeat > 1 else contextlib.nullcontext()
        with (
            loop_ctx,
            tc.tile_pool(name="persist", bufs=1) as pp,
            tc.tile_pool(name="e", bufs=8) as ep,
            tc.tile_pool(name="osb", bufs=4) as op_,
            tc.tile_pool(name="small", bufs=2) as sp,
        ):
            xp_sb = pp.tile([128, 2 * KD, T], f8)
            wqp_sb = pp.tile([128, 2 * KD, 2 * C], f8)
            wkp_sb = pp.tile([128, 2 * KD, C], f8)
            wvp_sb = pp.tile([128, 2 * KD, C], f8)
            wop_sb = pp.tile([128, 4, D], f8)
            x8_sb = xp_sb[:, 0:KD, :]
            dx8_sb = xp_sb[:, KD : 2 * KD, :]
            wq8_sb = wqp_sb[:, 0:KD, :]
            dwq8_sb = wqp_sb[:, KD : 2 * KD, :]
            wk8_sb = wkp_sb[:, 0:KD, :]
            dwk8_sb = wkp_sb[:, KD : 2 * KD, :]
            wv8_sb = wvp_sb[:, 0:KD, :]
            dwv8_sb = wvp_sb[:, KD : 2 * KD, :]
            wo8_sb = wop_sb[:, 0:2, :]
            dwo8_sb = wop_sb[:, 2:4, :]
            mk_sb = pp.tile([128, KT, T], f16)
            # per-head S operands: stationary k-stack [K8; dK8], moving
            # q-stack with (Q8, dQ8) slots duplicated across both halves
            kst_sb = pp.tile([128, NHC, 1, T], f8)
            qst_sb = pp.tile([128, NHC, 2, T], f8)
            v_sb = pp.tile([128, KT, NHC * 2 * DH], f16)
            hc8_sb = pp.tile([128, 2, T], f8)
            dhc8_sb = pp.tile([128, 2, T], f8)
            h16_sb = pp.tile([64, T], f16)
            k8t_sb = pp.tile([128, T], f8)
            dk8t_sb = pp.tile([128, T], f8)
            bq_sb = pp.tile([1, 2 * C], f16)
            bk_sb = pp.tile([1, C], f16)
            bv_sb = pp.tile([1, C], f16)
            ones512 = pp.tile([1, 512], f16)
            ones128 = pp.tile([1, 128], f16)

            # ---- input DMAs ----
            # wq pack + biases on SWDGE (its queue is otherwise idle at
            # startup); HWDGE carries x chunks with wk/wv packs slotted in
            # first-use order; masks and wo follow in the phase-1 epilogue.
            # only heads 0/1 weight columns gate the prefix; the rest of
            # the packs ride later in the stream so the mask load (behind
            # everything on the serialized DMA device) starts sooner
            nc.gpsimd.dma_start(out=wqp_sb[:, :, 0:256], in_=wqp_r[:, :, 0:256])
            nc.gpsimd.dma_start(out=bq_sb, in_=bq16[:, :])
            nc.gpsimd.dma_start(out=bk_sb, in_=bk16[:, :])
            nc.gpsimd.dma_start(out=bv_sb, in_=bv16[:, :])
            nc.gpsimd.dma_start(out=wqp_sb[:, :, 256:512], in_=wqp_r[:, :, 256:512])
            for ch in range(4):
                cs = slice(ch * 512, (ch + 1) * 512)
                nc.sync.dma_start(out=xp_sb[:, :, cs], in_=xp_r[:, :, cs])
                if ch == 0:
                    nc.sync.dma_start(out=wkp_sb, in_=wkp_r)
                elif ch == 1:
                    nc.sync.dma_start(out=wvp_sb, in_=wvp_r)
            nc.vector.memset(ones512, 1.0)
            nc.vector.memset(ones128, 1.0)
            v4 = v_sb.rearrange("p kt (h x) -> p kt h x", x=2 * DH)

            # ---- phase 1: projections (fp8 DR 3-term + fp16 bias) ----
            def proj_q_gen(h, chs, ceng=None):
                """Per-head Q^T block with host-duplicated weight columns:
                the psum is [q_h; q_h], so quantize + residual write the
                S moving stack directly (no partition-crossing DMAs)."""
                hs = slice(h * 128, (h + 1) * 128)
                for ch in chs:
                    cs = slice(ch * 512, (ch + 1) * 512)
                    pt = ps1.tile([128, 512], f32, tag="p")
                    for i in range(KD // 2):
                        ks = slice(2 * i, 2 * i + 2)
                        nc.tensor.matmul(
                            pt, wq8_sb[:, ks, hs], x8_sb[:, ks, cs],
                            start=(i == 0), stop=False, perf_mode=DR,
                        )
                        nc.tensor.matmul(
                            pt, dwq8_sb[:, ks, hs], x8_sb[:, ks, cs],
                            start=False, stop=False, perf_mode=DR,
                        )
                        nc.tensor.matmul(
                            pt, wq8_sb[:, ks, hs], dx8_sb[:, ks, cs],
                            start=False, stop=False, perf_mode=DR,
                        )
                        yield
                    nc.tensor.matmul(
                        pt, bq_sb[:, hs], ones512, start=False, stop=True
                    )
                    if ceng is nc.scalar:
                        nc.scalar.activation(
                            qst_sb[:, h, 0, cs], pt, AF.Identity, scale=SQK_PS
                        )
                    else:
                        nc.vector.tensor_scalar_mul(qst_sb[:, h, 0, cs], pt, SQK_PS)
                    nc.vector.scalar_tensor_tensor(
                        qst_sb[:, h, 1, cs], pt, SQK_PS, qst_sb[:, h, 0, cs],
                        mybir.AluOpType.mult, mybir.AluOpType.subtract,
                    )
                    yield

            def proj_k_gen(m, chs, do_asm, ceng=None):
                """Chunks of the K^T head-pair block (2m, 2m+1); quantize
                into persistent tmp tiles; after the last chunk, small
                SBUF->SBUF DMAs build the per-head stacks [K8; dK8]."""
                ms = slice(m * 128, (m + 1) * 128)
                for ch in chs:
                    cs = slice(ch * 512, (ch + 1) * 512)
                    pt = ps1.tile([128, 512], f32, tag="p")
                    for i in range(KD // 2):
                        ks = slice(2 * i, 2 * i + 2)
                        nc.tensor.matmul(
                            pt, wk8_sb[:, ks, ms], x8_sb[:, ks, cs],
                            start=(i == 0), stop=False, perf_mode=DR,
                        )
                        nc.tensor.matmul(
                            pt, dwk8_sb[:, ks, ms], x8_sb[:, ks, cs],
                            start=False, stop=False, perf_mode=DR,
                        )
                        nc.tensor.matmul(
                            pt, wk8_sb[:, ks, ms], dx8_sb[:, ks, cs],
                            start=False, stop=False, perf_mode=DR,
                        )
                        yield
                    nc.tensor.matmul(
                        pt, bk_sb[:, ms], ones512, start=False, stop=True
                    )
                    if ceng is nc.scalar:
                        nc.scalar.activation(
                            k8t_sb[:, cs], pt, AF.Identity, scale=SQK_PS
                        )
                    else:
                        nc.vector.tensor_scalar_mul(k8t_sb[:, cs], pt, SQK_PS)
                    nc.vector.scalar_tensor_tensor(
                        dk8t_sb[:, cs], pt, SQK_PS, k8t_sb[:, cs],
                        mybir.AluOpType.mult, mybir.AluOpType.subtract,
                    )
                    yield
                if do_asm:
                    for half in range(2):
                        h = 2 * m + half
                        srch = slice(half * 64, (half + 1) * 64)
                        nc.sync.dma_start(
                            out=kst_sb[0:64, h, 0, :], in_=k8t_sb[srch, :]
                        )
                        nc.sync.dma_start(
                            out=kst_sb[64:128, h, 0, :], in_=dk8t_sb[srch, :]
                        )
                    yield

            def proj_v_gen(ps1, tbs, ceng=None):
                # V: per 128-row t-block, kd-pair-packed DR 3-term
                for tb in tbs:
                    ts = slice(tb * 128, (tb + 1) * 128)
                    pvfull = ps1.tile([128, 512], f32, tag="p")
                    pv = pvfull[:, 0:C]
                    for i in range(KD // 2):
                        ks = slice(2 * i, 2 * i + 2)
                        nc.tensor.matmul(
                            pv, x8_sb[:, ks, ts], wv8_sb[:, ks, :],
                            start=(i == 0), stop=False, perf_mode=DR,
                        )
                        nc.tensor.matmul(
                            pv, dx8_sb[:, ks, ts], wv8_sb[:, ks, :],
                            start=False, stop=False, perf_mode=DR,
                        )
                        nc.tensor.matmul(
                            pv, x8_sb[:, ks, ts], dwv8_sb[:, ks, :],
                            start=False, stop=False, perf_mode=DR,
                        )
                    nc.tensor.matmul(pv, ones128, bv_sb, start=False, stop=True)
                    if ceng is nc.scalar:
                        nc.scalar.activation(
                            v4[:, tb, :, 0:DH],
                            pv.rearrange("p (h x) -> p h x", x=DH),
                            AF.Identity, scale=SV_PS,
                        )
                    else:
                        nc.vector.tensor_scalar_mul(
                            v4[:, tb, :, 0:DH],
                            pv.rearrange("p (h x) -> p h x", x=DH),
                            SV_PS,
                        )
                    yield

            # ---- phases 1+2+3 interleaved ----
            # Engines drain their queues in emission order, so phase 2 is
            # emitted as soon as heads 0/1 have operands (prefix below);
            # the rest of phase 1 (V blocks, m=1 projections) is injected
            # piecewise into the PE slack of the ACT-bound attention steps.
            # PSUM: pss 8K + psu 4K + (ps1 4K, later swapped for pso 4K).
            pss_cm = tc.tile_pool(name="ps_s", bufs=2, space="PSUM")
            pss = pss_cm.__enter__()
            psu_cm = tc.tile_pool(name="ps_u", bufs=1, space="PSUM")
            psu = psu_cm.__enter__()
            ps1_cm = tc.tile_pool(name="ps1", bufs=2, space="PSUM")
            ps1 = ps1_cm.__enter__()
            pso_cm = [None]
            pso = [None]
            if True:
                def phase3_gen(tbs, pool, use_act=False):
                    # ACT only once phase 2's exp stream has drained --
                    # a copy on ACT mid-phase-2 stretches the exp cadence
                    engines = (
                        (nc.scalar, nc.vector)
                        if use_act else (nc.vector,)
                    )
                    for tb in tbs:
                        ts = slice(tb * 128, (tb + 1) * 128)
                        ot = op_.tile([128, 1024], f16)
                        for ch in range(2):
                            cs = slice(ch * 512, (ch + 1) * 512)
                            po = pool.tile([128, 512], f32, tag="o")
                            nc.tensor.matmul(
                                po, hc8_sb[:, :, ts], wo8_sb[:, :, cs],
                                start=True, stop=False, perf_mode=DR,
                            )
                            nc.tensor.matmul(
                                po, dhc8_sb[:, :, ts], wo8_sb[:, :, cs],
                                start=False, stop=False, perf_mode=DR,
                            )
                            nc.tensor.matmul(
                                po, hc8_sb[:, :, ts], dwo8_sb[:, :, cs],
                                start=False, stop=True, perf_mode=DR,
                            )
                            eng = engines[(tb * 2 + ch) % len(engines)]
                            if eng is nc.scalar:
                                eng.activation(ot[:, cs], po, AF.Identity, scale=SOUT)
                            else:
                                eng.tensor_scalar_mul(ot[:, cs], po, SOUT)
                            if use_act:
                                # tail: one full-tile DMA (fewer triggers)
                                if ch == 1:
                                    nc.sync.dma_start(out=out[ts, :], in_=ot)
                            else:
                                nc.sync.dma_start(out=out[ts, cs], in_=ot[:, cs])
                            yield

                def s_matmul(h, kt, qh):
                    kst = kst_sb[:, h, 0:1, kt * 128 : (kt + 1) * 128]
                    kst = kst.broadcast_to((128, 2, 128))
                    st = pss.tile([128, 1024], f32, tag="s")
                    with tc.high_priority(offset=24):
                        for sub in range(2):
                            ch = qh * 2 + sub
                            nc.tensor.matmul(
                                st[:, sub * 512 : (sub + 1) * 512],
                                kst,
                                qst_sb[:, h, :, ch * 512 : (ch + 1) * 512],
                                start=True, stop=True, perf_mode=DR,
                            )
                    return st

                # -- PE p-state warmup: ~3.5us of junk matmuls on constant
                # data (no DMA deps, so they run at t~0 under the input
                # stream); every later matmul is then costed at full clock
                junk = ps1.tile([128, 512], f32, tag="p")
                for _ in range(12):
                    nc.tensor.matmul(junk, ones128, ones512, start=True, stop=True)

                # -- prefix: the minimum phase 1 before step (h0, qh0, 0) --
                nc.gpsimd.memset(v4[:, :, :, DH:], 1.0 / SH)
                for _ in proj_q_gen(0, range(4), nc.scalar):
                    pass
                for _ in proj_k_gen(0, range(4), True, nc.scalar):
                    pass
                for _ in proj_v_gen(ps1, range(8), nc.scalar):
                    pass
                for _ in proj_q_gen(1, range(2), nc.scalar):
                    pass

                # mask tiles: emitted on the in-order HWDGE queue AFTER the
                # k-m0 assembly DMAs so those don't wait behind 8MB of mask;
                # qh-major so tile (kt, qh) lands ahead of step (h0, qh, kt)
                # qh0 halves of kt-pairs first: ~1.02us of DMA per step of
                # demand in the first head's pass, so it never mask-stalls;
                # the qh1 halves stream long before step 32 needs them
                for qh in range(2):
                    qs = slice(qh * 1024, (qh + 1) * 1024)
                    for k2 in range(0, KT, 2):
                        nc.sync.dma_start(
                            out=mk_sb[:, k2 : k2 + 2, qs],
                            in_=mk_r[:, k2 : k2 + 2, qs],
                        )

                nc.sync.dma_start(out=wop_sb, in_=wop_r)

                # -- remaining phase 1, injected into the phase-2 steps in
                # deadline order (V-b[kt] before U[kt]; head operands before
                # their first attention block) --
                pending = [
                    proj_v_gen(ps1, range(8, KT)),
                    proj_q_gen(1, range(2, 4)),
                    proj_k_gen(1, range(4), True),
                    proj_q_gen(2, range(2)),
                    proj_q_gen(3, range(2)),
                    proj_q_gen(3, range(2, 4)),
                    proj_q_gen(2, range(2, 4)),
                ]

                def inject_piece(n=1):
                    for _ in range(n):
                        while pending:
                            try:
                                next(pending[0])
                                break
                            except StopIteration:
                                pending.pop(0)
                        if not pending:
                            break
                    if pending:
                        return
                    if pso[0] is None:
                        # all projections done: swap ps1's PSUM for ps_o so
                        # phase 3 can interleave with the remaining steps
                        ps1_cm.__exit__(None, None, None)
                        pso_cm[0] = tc.tile_pool(name="ps_o", bufs=2, space="PSUM")
                        pso[0] = pso_cm[0].__enter__()

                # head/q-half order: heads 0/1 first (their operands exist),
                # all q-half-0 blocks done by step 95 (phase 3a), and the
                # final block is an even head so its h-quant needs no
                # partition-shifting DMA before phase 3b
                blocks = [(0, 0), (1, 0), (0, 1), (1, 1),
                          (2, 0), (3, 0), (3, 1), (2, 1)]
                steps = [(h, qh, kt) for h, qh in blocks for kt in range(KT)]
                # S emitted two steps ahead: exp's operand is always ready
                # before ACT frees up, keeping the exp cadence semless
                s_q = [s_matmul(steps[0][0], steps[0][2], steps[0][1]),
                       s_matmul(steps[1][0], steps[1][2], steps[1][1])]
                u = None
                for i, (h, qh, kt) in enumerate(steps):
                    qs = slice(qh * 1024, (qh + 1) * 1024)
                    if kt == 0:
                        u = psu.tile([2 * DH, 1024], f32, tag="u")
                    st_cur = s_q.pop(0)
                    if i + 2 < len(steps):
                        nh, nqh, nkt = steps[i + 2]
                        s_q.append(s_matmul(nh, nkt, nqh))
                    e = ep.tile([128, 1024], f16)
                    nc.scalar.activation(e, st_cur, AF.Exp, scale=SEXP)
                    nc.vector.tensor_mul(e, e, mk_sb[:, kt, qs])
                    for sub in range(2):
                        nc.tensor.matmul(
                            u[:, sub * 512 : (sub + 1) * 512],
                            v_sb[:, kt, h * 2 * DH : (h + 1) * 2 * DH],
                            e[:, sub * 512 : (sub + 1) * 512],
                            start=(kt == 0),
                            stop=(kt == KT - 1),
                        )
                    inject_piece(1)
                    if kt == KT - 1:
                        with tc.high_priority(offset=40):
                            recb = sp.tile([64, 1024], f16, tag="recb")
                            with nc.allow_low_precision(
                                reason="h is re-quantized to an fp8 pair; "
                                "f16 reciprocal noise is far below that"
                            ):
                                nc.vector.reciprocal(recb, u[DH : 2 * DH, :])
                            nc.vector.tensor_mul(h16_sb[:, qs], u[0:DH, :], recb)
                            if h % 2 == 0:
                                # last block: DVE, it's on the critical path
                                # to phase 3b; earlier even heads on gpsimd
                                q_eng = nc.vector if i == len(steps) - 1 else nc.gpsimd
                                q_eng.tensor_copy(
                                    hc8_sb[0:64, h // 2, qs], h16_sb[:, qs]
                                )
                                q_eng.tensor_sub(
                                    dhc8_sb[0:64, h // 2, qs],
                                    h16_sb[:, qs],
                                    hc8_sb[0:64, h // 2, qs],
                                )
                            else:
                                h8t = sp.tile([64, 1024], f8, tag="h8t")
                                dh8t = sp.tile([64, 1024], f8, tag="dh8t")
                                nc.gpsimd.tensor_copy(h8t, h16_sb[:, qs])
                                nc.gpsimd.tensor_sub(dh8t, h16_sb[:, qs], h8t)
                                nc.sync.dma_start(
                                    out=hc8_sb[64:128, h // 2, qs], in_=h8t
                                )
                                nc.sync.dma_start(
                                    out=dhc8_sb[64:128, h // 2, qs], in_=dh8t
                                )
                    if i == steps.index((NHC - 1, 0, KT - 1)) + 6:
                        # q-half 0 of every head is normalized (2 steps ago,
                        # so its h-quant has landed and the first phase-3a
                        # matmul won't head-of-line-block the PE queue):
                        # inject the output projection of t-blocks 0..7
                        pending.append(phase3_gen(range(KT // 2), pso[0]))
                # drain any unfinished phase-3a work
                while pending:
                    inject_piece()
                # keep the PE busy through the last normalize/quant chain
                # so phase 3b is costed at full clock
                junk2 = pss.tile([128, 1024], f32, tag="s")
                for _ in range(10):
                    nc.tensor.matmul(
                        junk2[:, 0:512], ones128, ones512, start=True, stop=True
                    )
                # phase 2's PSUM is dead: hand it all to a deep phase-3b
                # pool so the last 8 blocks pipeline without psum stalls
                pso_cm[0].__exit__(None, None, None)
                if dbg:
                    nc.sync.dma_start(out=qst_o[:, :, :, :], in_=qst_sb)
                    nc.sync.dma_start(
                        out=kst_o[:, :, :], in_=kst_sb.rearrange("p h o t -> p (h o) t")
                    )
                    nc.sync.dma_start(out=v_o[:, :, :], in_=v_sb)
                    nc.sync.dma_start(out=hc_o[:, :, :], in_=hc8_sb)
                    nc.sync.dma_start(out=dhc_o[:, :, :], in_=dhc8_sb)
                psu_cm.__exit__(None, None, None)
                pss_cm.__exit__(None, None, None)
                with tc.tile_pool(name="ps_ob", bufs=6, space="PSUM") as psob:
                    for _ in phase3_gen(range(KT // 2, KT), psob, use_act=True):
                        pass
    nc.compile()
    return nc


def _get_nc(repeat=1):
    key = ("nc", repeat)
    if key not in _CACHE:
        _CACHE[key] = _build(repeat)
    return _CACHE[key]


def _pair(a):
    """fp8 value+residual decomposition of a float32 array."""
    a = np.asarray(a, np.float32)
    hi = a.astype(NP8)
    lo = (a - hi.astype(np.float32)).astype(NP8)
    return hi, lo


def _prep_core_inputs(c, x, mask, Wq, bq, Wk, bk, Wv, bv, Wo):
    b, hb = divmod(c, NCORES // B)
    sl = slice(hb * C, (hb + 1) * C)
    x8, dx8 = _pair(x[b].T * SX)
    wq_d = (Wq[:, sl] * SCALE * SWQ).reshape(D, NHC, DH)
    wq_d = np.concatenate([wq_d, wq_d], axis=2).reshape(D, 2 * C)
    bq_d = (bq[sl] * SCALE * SX * SWQ).reshape(NHC, DH)
    bq_d = np.concatenate([bq_d, bq_d], axis=1).reshape(2 * C)
    wq8, dwq8 = _pair(wq_d)
    wk8, dwk8 = _pair(Wk[:, sl] * SWK)
    wv8, dwv8 = _pair(Wv[:, sl] * SWK)
    wo8, dwo8 = _pair(Wo[sl, :] * SWK)
    return {
        "xp": np.ascontiguousarray(np.concatenate([x8, dx8], axis=0)),
        "wqp": np.ascontiguousarray(np.concatenate([wq8, dwq8], axis=0)),
        "wkp": np.ascontiguousarray(np.concatenate([wk8, dwk8], axis=0)),
        "wvp": np.ascontiguousarray(np.concatenate([wv8, dwv8], axis=0)),
        "wop": np.ascontiguousarray(np.concatenate([wo8, dwo8], axis=0)),
        "bq16": bq_d.astype(np.float16).reshape(1, 2 * C),
        "bk16": (bk[sl] * SX * SWK).astype(np.float16).reshape(1, C),
        "bv16": (bv[sl] * SX * SWK).astype(np.float16).reshape(1, C),
        "maskt": np.ascontiguousarray(mask[b].T).astype(np.float16),
    }


def kernel(
    inputs, mask, Wq, bq, Wk, bk, Wv, bv, Wo, bo,
    _trace=False, _trace_kwargs=None, _repeat=1,
):
    x = np.asarray(inputs, dtype=np.float32)
    mask = np.asarray(mask)
    Wq, bq = np.asarray(Wq, np.float32), np.asarray(bq, np.float32)
    Wk, bk = np.asarray(Wk, np.float32), np.asarray(bk, np.float32)
    Wv, bv = np.asarray(Wv, np.float32), np.asarray(bv, np.float32)
    Wo, bo = np.asarray(Wo, np.float32), np.asarray(bo, np.float32)

    nc = _get_nc(_repeat)
    in_maps = [
        _prep_core_inputs(c, x, mask, Wq, bq, Wk, bk, Wv, bv, Wo)
        for c in range(NCORES)
    ]
    last_err = None
    for attempt in range(3):
        try:
            res = run_bass_kernel_spmd(
                nc,
                in_maps,
                list(range(NCORES)),
                trace=_trace,
                **(_trace_kwargs or {}),
            )
            break
        except Exception as e:  # wedged device etc. -- retry
            last_err = e
            time.sleep(3.0)
    else:
        raise last_err
    out = np.empty((B, T, D), np.float32)
    per_b = NCORES // B
    for b in range(B):
        acc = res.results[b * per_b]["out"].astype(np.float32)
        for j in range(1, per_b):
            acc = acc + res.results[b * per_b + j]["out"].astype(np.float32)
        out[b] = acc + bo[None, :]
    if _trace:
        kernel.last_results = res
    return out


# revision 70
# speedup vs baseline: 1.1481x; 1.0000x over previous
"""Multi-head attention on 8 Trainium2 NeuronCores, fp8-DoubleRow edition.

Problem: B=2, T=2048, D=1024, H=16 heads (dh=64), int 0/1 attention mask.

Sharding (hardcoded): core c -> batch b = c//4, head block hb = c%4
(4 heads = 256 cols per core). Wq/Wk/Wv column-sharded, Wo row-sharded;
each core returns a partial [T, D] output, host sums the 4 partials per
batch and adds bo.

Precision strategy: every matmul except U (= attn @ V) runs as fp8e4
DoubleRow (0.5 cyc/row, 2 k-subtiles per instr) with full value+residual
correction, so accuracy stays fp16-class:
  A @ B == (A8 + dA8)(B8 + dB8) up to ~0.4% second-order residue, where
  X8 = fp8(X), dX8 = fp8(X - X8).
- projections (Q/K/V): 3-term DR (A8B8 | dA8.B8 | A8.dB8), kd-pair packed,
  x and W pairs prepped host-side; bias added via fp16 ones-matmul.
- S = K^T Q per head (contraction dh=64): ONE DR instr per output tile:
  stationary = [K8; dK8] stacked in partitions (128 rows), sub0 moving =
  [Q8; Q8], sub1 moving = [dQ8; dQ8]  =>  (K8+dK8)^T (Q8+dQ8) exactly.
- output proj: 3-term DR over the c-halves (contraction C=256 packed).
- U stays fp16: E = exp(S) can't be residual-corrected without an extra
  131k-row pass (ACT can only emit one output), and raw-fp8 E costs
  2.3e-2 rel err (> the 2e-2 gate).

Phase 2 runs per (head, q-half): S tiles [128,1024] double-buffered +
U accumulator [128,1024] fit PSUM exactly; exp is ACT-bound (~133us) and
everything else hides under it.  Engines drain their queues in emission
order, so the remaining phase-1 work (m=1 projections) is emitted
piecewise inside the step loop ("injections"), phase 3a (t-blocks 0-7)
is injected once every head's q-half 0 is normalized, and only phase 3b
trails the last attention block.  The denominator ones-columns carry
1/SH so the normalize multiply directly yields h*SH for the fp8 pair
(plain copy/sub, legal on gpsimd: GPSIMD cannot access PSUM, and Pool
TensorScalarPtr is rejected by codegen).

No max-subtraction is needed: scores are O(1) (exp range ~e^-6..e^6) and
softmax(x) == softmax(x - max) exactly in the masked-multiplicative form
E = exp(S) * m / sum(exp(S) * m).
"""
import contextlib
import os
import sys
import time

os.environ.setdefault("NEURON_RT_RESET_CORES", "1")

if "/opt/trn_rl_repo" not in sys.path:
    sys.path.insert(0, "/opt/trn_rl_repo")

import numpy as np
import ml_dtypes

import concourse.bass as bass  # noqa: F401  (import keeps bass registered)
from concourse import bacc
import concourse.mybir as mybir
import concourse.tile as tile
from concourse.bass_utils import run_bass_kernel_spmd

f32 = mybir.dt.float32
f16 = mybir.dt.float16
f8 = mybir.dt.float8e4
AF = mybir.ActivationFunctionType
DR = mybir.MatmulPerfMode.DoubleRow

NP8 = ml_dtypes.float8_e4m3

B, T, D, H = 2, 2048, 1024, 16
DH = 64                 # head dim
NHC = 4                 # heads per core
C = NHC * DH            # 256 columns per core
KD = D // 128           # 8 contraction tiles over D
KT = T // 128           # 16 k-tiles over T
NCORES = 8
SCALE = DH ** -0.5      # 0.125

# power-of-two fp8 range scaling: e4m3 normals span ~[2**-6, 240], and the
# raw tensors here (W ~ N(0, 1/D), q ~ 0.125, h ~ 0.05) sit deep in its
# subnormal floor, which destroys the value+residual pairs.  Each fp8
# tensor is stored pre-scaled into e4m3's sweet spot; the compensations
# ride existing scale slots (ACT activation scale, DVE/gpsimd
# tensor_scalar ops), so they cost nothing extra.
SX = 2.0 ** 5        # x
SWQ = 2.0 ** 13      # Wq * SCALE
SWK = 2.0 ** 10      # Wk / Wv / Wo
SQ = 2.0 ** 8        # q pair
SK = 2.0 ** 5        # k pair
SH = 2.0 ** 8        # h pair
SQK_PS = 2.0 ** -10  # psum(q*2^18) -> q*2^8; psum(k*2^15) -> k*2^5
SV_PS = 2.0 ** -15   # psum(v*2^15) -> v
SEXP = 2.0 ** -13    # S psum carries scores * 2^(8+5)
SOUT = 2.0 ** -18    # phase-3 psum carries out * 2^(8+10)

_CACHE = {}


def _build(repeat=1, dbg=False):
    nc = bacc.Bacc()
    xp = nc.declare_dram_parameter("xp", [2 * D, T], f8, isOutput=False)
    wqp = nc.declare_dram_parameter("wqp", [2 * D, 2 * C], f8, isOutput=False)
    wkp = nc.declare_dram_parameter("wkp", [2 * D, C], f8, isOutput=False)
    wvp = nc.declare_dram_parameter("wvp", [2 * D, C], f8, isOutput=False)
    wop = nc.declare_dram_parameter("wop", [2 * C, D], f8, isOutput=False)
    bq16 = nc.declare_dram_parameter("bq16", [1, 2 * C], f16, isOutput=False)
    bk16 = nc.declare_dram_parameter("bk16", [1, C], f16, isOutput=False)
    bv16 = nc.declare_dram_parameter("bv16", [1, C], f16, isOutput=False)
    maskt = nc.declare_dram_parameter("maskt", [T, T], f16, isOutput=False)
    out = nc.declare_dram_parameter("out", [T, D], f16, isOutput=True)
    if dbg:
        qst_o = nc.declare_dram_parameter("qst_o", [128, NHC, 2, T], f8, isOutput=True)
        kst_o = nc.declare_dram_parameter("kst_o", [128, NHC, T], f8, isOutput=True)
        v_o = nc.declare_dram_parameter("v_o", [128, KT, NHC * 2 * DH], f16, isOutput=True)
        hc_o = nc.declare_dram_parameter("hc_o", [128, 2, T], f8, isOutput=True)
        dhc_o = nc.declare_dram_parameter("dhc_o", [128, 2, T], f8, isOutput=True)

    xp_r = xp.rearrange("(g kd p) t -> p (g kd) t", p=128, g=2)
    wqp_r = wqp.rearrange("(g kd p) c -> p (g kd) c", p=128, g=2)
    wkp_r = wkp.rearrange("(g kd p) c -> p (g kd) c", p=128, g=2)
    wvp_r = wvp.rearrange("(g kd p) c -> p (g kd) c", p=128, g=2)
    wop_r = wop.rearrange("(g h p) d -> p (g h) d", p=128, g=2)
    mk_r = maskt.rearrange("(kt p) t -> p kt t", p=128)

    with tile.TileContext(nc) as tc:
        loop_ctx = tc.For_i(0, repeat, 1) if repeat > 1 else contextlib.nullcontext()
        with (
            loop_ctx,
            tc.tile_pool(name="persist", bufs=1) as pp,
            tc.tile_pool(name="e", bufs=8) as ep,
            tc.tile_pool(name="osb", bufs=4) as op_,
            tc.tile_pool(name="small", bufs=2) as sp,
        ):
            xp_sb = pp.tile([128, 2 * KD, T], f8)
            wqp_sb = pp.tile([128, 2 * KD, 2 * C], f8)
            wkp_sb = pp.tile([128, 2 * KD, C], f8)
            wvp_sb = pp.tile([128, 2 * KD, C], f8)
            wop_sb = pp.tile([128, 4, D], f8)
            x8_sb = xp_sb[:, 0:KD, :]
            dx8_sb = xp_sb[:, KD : 2 * KD, :]
            wq8_sb = wqp_sb[:, 0:KD, :]
            dwq8_sb = wqp_sb[:, KD : 2 * KD, :]
            wk8_sb = wkp_sb[:, 0:KD, :]
            dwk8_sb = wkp_sb[:, KD : 2 * KD, :]
            wv8_sb = wvp_sb[:, 0:KD, :]
            dwv8_sb = wvp_sb[:, KD : 2 * KD, :]
            wo8_sb = wop_sb[:, 0:2, :]
            dwo8_sb = wop_sb[:, 2:4, :]
            mk_sb = pp.tile([128, KT, T], f16)
            # per-head S operands: stationary k-stack [K8; dK8], moving
            # q-stack with (Q8, dQ8) slots duplicated across both halves
            kst_sb = pp.tile([128, NHC, 1, T], f8)
            qst_sb = pp.tile([128, NHC, 2, T], f8)
            v_sb = pp.tile([128, KT, NHC * 2 * DH], f16)
            hc8_sb = pp.tile([128, 2, T], f8)
            dhc8_sb = pp.tile([128, 2, T], f8)
            h16_sb = pp.tile([64, T], f16)
            k8t_sb = pp.tile([128, T], f8)
            dk8t_sb = pp.tile([128, T], f8)
            bq_sb = pp.tile([1, 2 * C], f16)
            bk_sb = pp.tile([1, C], f16)
            bv_sb = pp.tile([1, C], f16)
            ones512 = pp.tile([1, 512], f16)
            ones128 = pp.tile([1, 128], f16)

            # ---- input DMAs ----
            # wq pack + biases on SWDGE (its queue is otherwise idle at
            # startup); HWDGE carries x chunks with wk/wv packs slotted in
            # first-use order; masks and wo follow in the phase-1 epilogue.
            # only heads 0/1 weight columns gate the prefix; the rest of
            # the packs ride later in the stream so the mask load (behind
            # everything on the serialized DMA device) starts sooner
            nc.gpsimd.dma_start(out=wqp_sb[:, :, 0:256], in_=wqp_r[:, :, 0:256])
            nc.gpsimd.dma_start(out=bq_sb, in_=bq16[:, :])
            nc.gpsimd.dma_start(out=bk_sb, in_=bk16[:, :])
            nc.gpsimd.dma_start(out=bv_sb, in_=bv16[:, :])
            nc.gpsimd.dma_start(out=wqp_sb[:, :, 256:512], in_=wqp_r[:, :, 256:512])
            for ch in range(4):
                cs = slice(ch * 512, (ch + 1) * 512)
                nc.sync.dma_start(out=xp_sb[:, :, cs], in_=xp_r[:, :, cs])
                if ch == 0:
                    nc.sync.dma_start(out=wkp_sb, in_=wkp_r)
                elif ch == 1:
                    nc.sync.dma_start(out=wvp_sb, in_=wvp_r)
            nc.vector.memset(ones512, 1.0)
            nc.vector.memset(ones128, 1.0)
            v4 = v_sb.rearrange("p kt (h x) -> p kt h x", x=2 * DH)

            # ---- phase 1: projections (fp8 DR 3-term + fp16 bias) ----
            def proj_q_gen(h, chs, ceng=None):
                """Per-head Q^T block with host-duplicated weight columns:
                the psum is [q_h; q_h], so quantize + residual write the
                S moving stack directly (no partition-crossing DMAs)."""
                hs = slice(h * 128, (h + 1) * 128)
                for ch in chs:
                    cs = slice(ch * 512, (ch + 1) * 512)
                    pt = ps1.tile([128, 512], f32, tag="p")
                    for i in range(KD // 2):
                        ks = slice(2 * i, 2 * i + 2)
                        nc.tensor.matmul(
                            pt, wq8_sb[:, ks, hs], x8_sb[:, ks, cs],
                            start=(i == 0), stop=False, perf_mode=DR,
                        )
                        nc.tensor.matmul(
                            pt, dwq8_sb[:, ks, hs], x8_sb[:, ks, cs],
                            start=False, stop=False, perf_mode=DR,
                        )
                        nc.tensor.matmul(
                            pt, wq8_sb[:, ks, hs], dx8_sb[:, ks, cs],
                            start=False, stop=False, perf_mode=DR,
                        )
                        yield
                    nc.tensor.matmul(
                        pt, bq_sb[:, hs], ones512, start=False, stop=True
                    )
                    if ceng is nc.scalar:
                        nc.scalar.activation(
                            qst_sb[:, h, 0, cs], pt, AF.Identity, scale=SQK_PS
                        )
                    else:
                        nc.vector.tensor_scalar_mul(qst_sb[:, h, 0, cs], pt, SQK_PS)
                    nc.vector.scalar_tensor_tensor(
                        qst_sb[:, h, 1, cs], pt, SQK_PS, qst_sb[:, h, 0, cs],
                        mybir.AluOpType.mult, mybir.AluOpType.subtract,
                    )
                    yield

            def proj_k_gen(m, chs, do_asm, ceng=None):
                """Chunks of the K^T head-pair block (2m, 2m+1); quantize
                into persistent tmp tiles; after the last chunk, small
                SBUF->SBUF DMAs build the per-head stacks [K8; dK8]."""
                ms = slice(m * 128, (m + 1) * 128)
                for ch in chs:
                    cs = slice(ch * 512, (ch + 1) * 512)
                    pt = ps1.tile([128, 512], f32, tag="p")
                    for i in range(KD // 2):
                        ks = slice(2 * i, 2 * i + 2)
                        nc.tensor.matmul(
                            pt, wk8_sb[:, ks, ms], x8_sb[:, ks, cs],
                            start=(i == 0), stop=False, perf_mode=DR,
                        )
                        nc.tensor.matmul(
                            pt, dwk8_sb[:, ks, ms], x8_sb[:, ks, cs],
                            start=False, stop=False, perf_mode=DR,
                        )
                        nc.tensor.matmul(
                            pt, wk8_sb[:, ks, ms], dx8_sb[:, ks, cs],
                            start=False, stop=False, perf_mode=DR,
                        )
                        yield
                    nc.tensor.matmul(
                        pt, bk_sb[:, ms], ones512, start=False, stop=True
                    )
                    if ceng is nc.scalar:
                        nc.scalar.activation(
                            k8t_sb[:, cs], pt, AF.Identity, scale=SQK_PS
                        )
                    else:
                        nc.vector.tensor_scalar_mul(k8t_sb[:, cs], pt, SQK_PS)
                    nc.vector.scalar_tensor_tensor(
                        dk8t_sb[:, cs], pt, SQK_PS, k8t_sb[:, cs],
                        mybir.AluOpType.mult, mybir.AluOpType.subtract,
                    )
                    yield
                if do_asm:
                    for half in range(2):
                        h = 2 * m + half
                        srch = slice(half * 64, (half + 1) * 64)
                        nc.sync.dma_start(
                            out=kst_sb[0:64, h, 0, :], in_=k8t_sb[srch, :]
                        )
                        nc.sync.dma_start(
                            out=kst_sb[64:128, h, 0, :], in_=dk8t_sb[srch, :]
                        )
                    yield

            def proj_v_gen(ps1, tbs, ceng=None):
                # V: per 128-row t-block, kd-pair-packed DR 3-term
                for tb in tbs:
                    ts = slice(tb * 128, (tb + 1) * 128)
                    pvfull = ps1.tile([128, 512], f32, tag="p")
                    pv = pvfull[:, 0:C]
                    for i in range(KD // 2):
                        ks = slice(2 * i, 2 * i + 2)
                        nc.tensor.matmul(
                            pv, x8_sb[:, ks, ts], wv8_sb[:, ks, :],
                            start=(i == 0), stop=False, perf_mode=DR,
                        )
                        nc.tensor.matmul(
                            pv, dx8_sb[:, ks, ts], wv8_sb[:, ks, :],
                            start=False, stop=False, perf_mode=DR,
                        )
                        nc.tensor.matmul(
                            pv, x8_sb[:, ks, ts], dwv8_sb[:, ks, :],
                            start=False, stop=False, perf_mode=DR,
                        )
                    nc.tensor.matmul(pv, ones128, bv_sb, start=False, stop=True)
                    if ceng is nc.scalar:
                        nc.scalar.activation(
                            v4[:, tb, :, 0:DH],
                            pv.rearrange("p (h x) -> p h x", x=DH),
                            AF.Identity, scale=SV_PS,
                        )
                    else:
                        nc.vector.tensor_scalar_mul(
                            v4[:, tb, :, 0:DH],
                            pv.rearrange("p (h x) -> p h x", x=DH),
                            SV_PS,
                        )
                    yield

            # ---- phases 1+2+3 interleaved ----
            # Engines drain their queues in emission order, so phase 2 is
            # emitted as soon as heads 0/1 have operands (prefix below);
            # the rest of phase 1 (V blocks, m=1 projections) is injected
            # piecewise into the PE slack of the ACT-bound attention steps.
            # PSUM: pss 8K + psu 4K + (ps1 4K, later swapped for pso 4K).
            pss_cm = tc.tile_pool(name="ps_s", bufs=2, space="PSUM")
            pss = pss_cm.__enter__()
            psu_cm = tc.tile_pool(name="ps_u", bufs=1, space="PSUM")
            psu = psu_cm.__enter__()
            ps1_cm = tc.tile_pool(name="ps1", bufs=2, space="PSUM")
            ps1 = ps1_cm.__enter__()
            pso_cm = [None]
            pso = [None]
            if True:
                def phase3_gen(tbs, pool, use_act=False):
                    # ACT only once phase 2's exp stream has drained --
                    # a copy on ACT mid-phase-2 stretches the exp cadence
                    engines = (
                        (nc.scalar, nc.vector)
                        if use_act else (nc.vector,)
                    )
                    for tb in tbs:
                        ts = slice(tb * 128, (tb + 1) * 128)
                        ot = op_.tile([128, 1024], f16)
                        for ch in range(2):
                            cs = slice(ch * 512, (ch + 1) * 512)
                            po = pool.tile([128, 512], f32, tag="o")
                            nc.tensor.matmul(
                                po, hc8_sb[:, :, ts], wo8_sb[:, :, cs],
                                start=True, stop=False, perf_mode=DR,
                            )
                            nc.tensor.matmul(
                                po, dhc8_sb[:, :, ts], wo8_sb[:, :, cs],
                                start=False, stop=False, perf_mode=DR,
                            )
                            nc.tensor.matmul(
                                po, hc8_sb[:, :, ts], dwo8_sb[:, :, cs],
                                start=False, stop=True, perf_mode=DR,
                            )
                            eng = engines[(tb * 2 + ch) % len(engines)]
                            if eng is nc.scalar:
                                eng.activation(ot[:, cs], po, AF.Identity, scale=SOUT)
                            else:
                                eng.tensor_scalar_mul(ot[:, cs], po, SOUT)
                            if use_act:
                                # tail: one full-tile DMA (fewer triggers)
                                if ch == 1:
                                    nc.sync.dma_start(out=out[ts, :], in_=ot)
                            else:
                                nc.sync.dma_start(out=out[ts, cs], in_=ot[:, cs])
                            yield

                def s_matmul(h, kt, qh):
                    kst = kst_sb[:, h, 0:1, kt * 128 : (kt + 1) * 128]
                    kst = kst.broadcast_to((128, 2, 128))
                    st = pss.tile([128, 1024], f32, tag="s")
                    with tc.high_priority(offset=24):
                        for sub in range(2):
                            ch = qh * 2 + sub
                            nc.tensor.matmul(
                                st[:, sub * 512 : (sub + 1) * 512],
                                kst,
                                qst_sb[:, h, :, ch * 512 : (ch + 1) * 512],
                                start=True, stop=True, perf_mode=DR,
                            )
                    return st

                # -- PE p-state warmup: ~3.5us of junk matmuls on constant
                # data (no DMA deps, so they run at t~0 under the input
                # stream); every later matmul is then costed at full clock
                junk = ps1.tile([128, 512], f32, tag="p")
                for _ in range(12):
                    nc.tensor.matmul(junk, ones128, ones512, start=True, stop=True)

                # -- prefix: the minimum phase 1 before step (h0, qh0, 0) --
                nc.gpsimd.memset(v4[:, :, :, DH:], 1.0 / SH)
                for _ in proj_q_gen(0, range(4), nc.scalar):
                    pass
                for _ in proj_k_gen(0, range(4), True, nc.scalar):
                    pass
                for _ in proj_v_gen(ps1, range(8), nc.scalar):
                    pass
                for _ in proj_q_gen(1, range(2), nc.scalar):
                    pass

                # mask tiles: emitted on the in-order HWDGE queue AFTER the
                # k-m0 assembly DMAs so those don't wait behind 8MB of mask;
                # qh-major so tile (kt, qh) lands ahead of step (h0, qh, kt)
                # qh0 halves of kt-pairs first: ~1.02us of DMA per step of
                # demand in the first head's pass, so it never mask-stalls;
                # the qh1 halves stream long before step 32 needs them
                for qh in range(2):
                    qs = slice(qh * 1024, (qh + 1) * 1024)
                    for k2 in range(0, KT, 2):
                        nc.sync.dma_start(
                            out=mk_sb[:, k2 : k2 + 2, qs],
                            in_=mk_r[:, k2 : k2 + 2, qs],
                        )

                nc.sync.dma_start(out=wop_sb, in_=wop_r)

                # -- remaining phase 1, injected into the phase-2 steps in
                # deadline order (V-b[kt] before U[kt]; head operands before
                # their first attention block) --
                pending = [
                    proj_v_gen(ps1, range(8, KT)),
                    proj_q_gen(1, range(2, 4)),
                    proj_k_gen(1, range(4), True),
                    proj_q_gen(2, range(2)),
                    proj_q_gen(3, range(2)),
                    proj_q_gen(3, range(2, 4)),
                    proj_q_gen(2, range(2, 4)),
                ]

                def inject_piece(n=1):
                    for _ in range(n):
                        while pending:
                            try:
                                next(pending[0])
                                break
                            except StopIteration:
                                pending.pop(0)
                        if not pending:
                            break
                    if pending:
                        return
                    if pso[0] is None:
                        # all projections done: swap ps1's PSUM for ps_o so
                        # phase 3 can interleave with the remaining steps
                        ps1_cm.__exit__(None, None, None)
                        pso_cm[0] = tc.tile_pool(name="ps_o", bufs=2, space="PSUM")
                        pso[0] = pso_cm[0].__enter__()

                # head/q-half order: heads 0/1 first (their operands exist),
                # all q-half-0 blocks done by step 95 (phase 3a), and the
                # final block is an even head so its h-quant needs no
                # partition-shifting DMA before phase 3b
                blocks = [(0, 0), (1, 0), (0, 1), (1, 1),
                          (2, 0), (3, 0), (3, 1), (2, 1)]
                steps = [(h, qh, kt) for h, qh in blocks for kt in range(KT)]
                # S emitted two steps ahead: exp's operand is always ready
                # before ACT frees up, keeping the exp cadence semless
                s_q = [s_matmul(steps[0][0], steps[0][2], steps[0][1]),
                       s_matmul(steps[1][0], steps[1][2], steps[1][1])]
                u = None
                for i, (h, qh, kt) in enumerate(steps):
                    qs = slice(qh * 1024, (qh + 1) * 1024)
                    if kt == 0:
                        u = psu.tile([2 * DH, 1024], f32, tag="u")
                    st_cur = s_q.pop(0)
                    if i + 2 < len(steps):
                        nh, nqh, nkt = steps[i + 2]
                        s_q.append(s_matmul(nh, nkt, nqh))
                    e = ep.tile([128, 1024], f16)
                    nc.scalar.activation(e, st_cur, AF.Exp, scale=SEXP)
                    nc.vector.tensor_mul(e, e, mk_sb[:, kt, qs])
                    for sub in range(2):
                        nc.tensor.matmul(
                            u[:, sub * 512 : (sub + 1) * 512],
                            v_sb[:, kt, h * 2 * DH : (h + 1) * 2 * DH],
                            e[:, sub * 512 : (sub + 1) * 512],
                            start=(kt == 0),
                            stop=(kt == KT - 1),
                        )
                    inject_piece(1)
                    if kt == KT - 1:
                        with tc.high_priority(offset=40):
                            recb = sp.tile([64, 1024], f16, tag="recb")
                            with nc.allow_low_precision(
                                reason="h is re-quantized to an fp8 pair; "
                                "f16 reciprocal noise is far below that"
                            ):
                                nc.vector.reciprocal(recb, u[DH : 2 * DH, :])
                            nc.vector.tensor_mul(h16_sb[:, qs], u[0:DH, :], recb)
                            if h % 2 == 0:
                                # last block: DVE, it's on the critical path
                                # to phase 3b; earlier even heads on gpsimd
                                q_eng = nc.vector if i == len(steps) - 1 else nc.gpsimd
                                q_eng.tensor_copy(
                                    hc8_sb[0:64, h // 2, qs], h16_sb[:, qs]
                                )
                                q_eng.tensor_sub(
                                    dhc8_sb[0:64, h // 2, qs],
                                    h16_sb[:, qs],
                                    hc8_sb[0:64, h // 2, qs],
                                )
                            else:
                                h8t = sp.tile([64, 1024], f8, tag="h8t")
                                dh8t = sp.tile([64, 1024], f8, tag="dh8t")
                                nc.gpsimd.tensor_copy(h8t, h16_sb[:, qs])
                                nc.gpsimd.tensor_sub(dh8t, h16_sb[:, qs], h8t)
                                nc.sync.dma_start(
                                    out=hc8_sb[64:128, h // 2, qs], in_=h8t
                                )
                                nc.sync.dma_start(
                                    out=dhc8_sb[64:128, h // 2, qs], in_=dh8t
                                )
                    if i == steps.index((NHC - 1, 0, KT - 1)) + 6:
                        # q-half 0 of every head is normalized (2 steps ago,
                        # so its h-quant has landed and the first phase-3a
                        # matmul won't head-of-line-block the PE queue):
                        # inject the output projection of t-blocks 0..7
                        pending.append(phase3_gen(range(KT // 2), pso[0]))
                # drain any unfinished phase-3a work
                while pending:
                    inject_piece()
                # keep the PE busy through the last normalize/quant chain
                # so phase 3b is costed at full clock
                junk2 = pss.tile([128, 1024], f32, tag="s")
                for _ in range(10):
                    nc.tensor.matmul(
                        junk2[:, 0:512], ones128, ones512, start=True, stop=True
                    )
                # phase 2's PSUM is dead: hand it all to a deep phase-3b
                # pool so the last 8 blocks pipeline without psum stalls
                pso_cm[0].__exit__(None, None, None)
                if dbg:
                    nc.sync.dma_start(out=qst_o[:, :, :, :], in_=qst_sb)
                    nc.sync.dma_start(
                        out=kst_o[:, :, :], in_=kst_sb.rearrange("p h o t -> p (h o) t")
                    )
                    nc.sync.dma_start(out=v_o[:, :, :], in_=v_sb)
                    nc.sync.dma_start(out=hc_o[:, :, :], in_=hc8_sb)
                    nc.sync.dma_start(out=dhc_o[:, :, :], in_=dhc8_sb)
                psu_cm.__exit__(None, None, None)
                pss_cm.__exit__(None, None, None)
                with tc.tile_pool(name="ps_ob", bufs=8, space="PSUM") as psob:
                    for _ in phase3_gen(range(KT // 2, KT), psob, use_act=True):
                        pass
    nc.compile()
    return nc


def _get_nc(repeat=1):
    key = ("nc", repeat)
    if key not in _CACHE:
        _CACHE[key] = _build(repeat)
    return _CACHE[key]


def _pair(a):
    """fp8 value+residual decomposition of a float32 array."""
    a = np.asarray(a, np.float32)
    hi = a.astype(NP8)
    lo = (a - hi.astype(np.float32)).astype(NP8)
    return hi, lo


def _prep_core_inputs(c, x, mask, Wq, bq, Wk, bk, Wv, bv, Wo):
    b, hb = divmod(c, NCORES // B)
    sl = slice(hb * C, (hb + 1) * C)
    x8, dx8 = _pair(x[b].T * SX)
    wq_d = (Wq[:, sl] * SCALE * SWQ).reshape(D, NHC, DH)
    wq_d = np.concatenate([wq_d, wq_d], axis=2).reshape(D, 2 * C)
    bq_d = (bq[sl] * SCALE * SX * SWQ).reshape(NHC, DH)
    bq_d = np.concatenate([bq_d, bq_d], axis=1).reshape(2 * C)
    wq8, dwq8 = _pair(wq_d)
    wk8, dwk8 = _pair(Wk[:, sl] * SWK)
    wv8, dwv8 = _pair(Wv[:, sl] * SWK)
    wo8, dwo8 = _pair(Wo[sl, :] * SWK)
    return {
        "xp": np.ascontiguousarray(np.concatenate([x8, dx8], axis=0)),
        "wqp": np.ascontiguousarray(np.concatenate([wq8, dwq8], axis=0)),
        "wkp": np.ascontiguousarray(np.concatenate([wk8, dwk8], axis=0)),
        "wvp": np.ascontiguousarray(np.concatenate([wv8, dwv8], axis=0)),
        "wop": np.ascontiguousarray(np.concatenate([wo8, dwo8], axis=0)),
        "bq16": bq_d.astype(np.float16).reshape(1, 2 * C),
        "bk16": (bk[sl] * SX * SWK).astype(np.float16).reshape(1, C),
        "bv16": (bv[sl] * SX * SWK).astype(np.float16).reshape(1, C),
        "maskt": np.ascontiguousarray(mask[b].T).astype(np.float16),
    }


def kernel(
    inputs, mask, Wq, bq, Wk, bk, Wv, bv, Wo, bo,
    _trace=False, _trace_kwargs=None, _repeat=1,
):
    x = np.asarray(inputs, dtype=np.float32)
    mask = np.asarray(mask)
    Wq, bq = np.asarray(Wq, np.float32), np.asarray(bq, np.float32)
    Wk, bk = np.asarray(Wk, np.float32), np.asarray(bk, np.float32)
    Wv, bv = np.asarray(Wv, np.float32), np.asarray(bv, np.float32)
    Wo, bo = np.asarray(Wo, np.float32), np.asarray(bo, np.float32)

    nc = _get_nc(_repeat)
    in_maps = [
        _prep_core_inputs(c, x, mask, Wq, bq, Wk, bk, Wv, bv, Wo)
        for c in range(NCORES)
    ]
    last_err = None
    for attempt in range(3):
        try:
            res = run_bass_kernel_spmd(
                nc,
                in_maps,
                list(range(NCORES)),
                trace=_trace,
                **(_trace_kwargs or {}),
            )
            break
        except Exception as e:  # wedged device etc. -- retry
            last_err = e
            time.sleep(3.0)
    else:
        raise last_err
    out = np.empty((B, T, D), np.float32)
    per_b = NCORES // B
    for b in range(B):
        acc = res.results[b * per_b]["out"].astype(np.float32)
        for j in range(1, per_b):
            acc = acc + res.results[b * per_b + j]["out"].astype(np.float32)
        out[b] = acc + bo[None, :]
    if _trace:
        kernel.last_results = res
    return out
